# revision 1
# baseline (speedup 1.0000x reference)
"""Block-diagonal grouped GEMM (GroupLinear) on 8 TRN2 NeuronCores.

Problem: x [8, 2048, 4096] f32, W [4096, 4096] f32 where only the 64
diagonal 64x64 blocks of W are used:
    y[b,s, g*64+o] = sum_i x[b,s, g*64+i] * W[g*64+o, g*64+i]

The kernel is HBM-bandwidth bound (per-NC cap ~358 GB/s; every x element
is read once, every y element written once). The correctness budget
(rel err < 2e-2) is far looser than f16 rounding (~3e-4), so all device
traffic is f16: 16MB x-in + 16MB y-out + 1MB weights per core instead
of the 66MB an f32 kernel moves -> ~2x.

Strategy:
  - Data-parallel over batch: core b handles x[b] (2048 tokens).
  - Host packs x[b].T into strip-major layout xp [128, 32*2048] f16 so
    every load is one contiguous 0.5-1MB DMA (4-8KB per partition line).
  - Two 64-ch groups pack into one 128-wide block-diagonal weight strip
    [128i, 128o]; 32 strips resident in SBUF (1MB f16).
  - Per chunk (1-2 strips): load, matmuls [K=128]x[128,512] into 2-bank
    [128,1024] PSUM tiles, one 1024-wide PSUM->SBUF f16 cast per tile
    alternating Vector/Scalar, store. Weights stay zero-padded in DRAM:
    a compact-load + on-device expansion variant saved 1.2us of DMA but
    cost 3-4us of pipeline fill (expansion copies serialize with chunk
    0's casts or get scheduler-reordered behind big memsets) -- net loss.
  - Loads ride the Sync HWDGE ring (all emitted first, so a store that
    waits on compute semaphores can never block a later load -- HWDGE
    DMAs are FIFO per issuing engine; loads must NOT ride the Scalar
    store ring even during fill: +10us measured). The two leading
    chunks are single-strip so compute starts early; the last four
    chunks store in 512KB halves (alternating Sync/Scalar rings) to
    shorten the drain tail.
  - Host unpacks/upcasts y. All device DMAs are perfectly contiguous.
"""

import numpy as np

import concourse.bacc as bacc
import concourse.mybir as mybir
from concourse.tile import TileContext
from concourse.bass_utils import run_bass_kernel_spmd

B, S, C = 8, 2048, 4096
G, GS = 64, 64            # groups, group size (=in_scale=out_scale)
NSTRIP = C // 128         # 32 strips of 128 channels (2 groups each)
TOK = 512                 # matmul moving free dim (PSUM bank = 512 f32)
PB = 1024                 # psum tile width (2 banks), one copy per tile
F16 = mybir.dt.float16
FP32 = mybir.dt.float32

# (start_strip, n_strips) chunks: two single-strip leaders for a short
# pipeline fill, then 1MB double-strip chunks.
CHUNKS = [(0, 1), (1, 1)] + [(c, 2) for c in range(2, NSTRIP, 2)]


def _build_program():
    nc = bacc.Bacc()
    xp = nc.declare_dram_parameter("xp", [128, NSTRIP * S], F16, isOutput=False)
    wb = nc.declare_dram_parameter("wb", [128, NSTRIP * 128], F16, isOutput=False)
    yp = nc.declare_dram_parameter("yp", [128, NSTRIP * S], F16, isOutput=True)

    with TileContext(nc) as tc:
        with (
            tc.tile_pool(name="wpool", bufs=1) as wpool,
            # The two single-strip leader chunks get dedicated pools:
            # mixing 512KB and 1MB tiles in one ring makes the first 1MB
            # tile alias the leaders' memory, so its load carries a WAR
            # dependency on their compute -- prefetch collapses and the
            # whole pipeline convoys for ~5us (seen in trace as load #3
            # issuing right after chunk 1's last matmul).
            tc.tile_pool(name="xlead", bufs=2) as xlead,
            tc.tile_pool(name="olead", bufs=2) as olead,
            tc.tile_pool(name="xpool", bufs=3) as xpool,
            tc.tile_pool(name="opool", bufs=3) as opool,
            tc.tile_pool(name="ppool", bufs=4, space="PSUM") as ppool,
        ):
            # Weights ride the Scalar (store) ring, which is idle during
            # fill. Strips 0-1 come in a small leading DMA so matmul 0
            # isn't gated on the full 1MB.
            w_sb = wpool.tile([128, NSTRIP * 128], F16)
            nc.scalar.dma_start(out=w_sb[:, :256], in_=wb[:, :256])
            nc.scalar.dma_start(out=w_sb[:, 256:], in_=wb[:, 256:])

            # Emit every load first: the Sync engine's queue is then all
            # loads (paced by xpool buffer reuse), so a drain-phase store
            # issued on Sync can never block a later load (HWDGE DMAs are
            # FIFO per issuing engine). xpool bufs still bounds prefetch.
            x_tiles = []
            for ci, (c0, ns) in enumerate(CHUNKS):
                x_t = (xlead if ci < 2 else xpool).tile([128, ns * S], F16)
                nc.sync.dma_start(
                    out=x_t[:], in_=xp[:, c0 * S : c0 * S + ns * S]
                )
                x_tiles.append(x_t)

            ncopy = 0
            for ci, (c0, ns) in enumerate(CHUNKS):
                cw = ns * S
                x_t = x_tiles[ci]
                o_t = (olead if ci < 2 else opool).tile([128, cw], F16)
                for pb in range(cw // PB):
                    s, half = divmod(pb, 2)
                    ps = ppool.tile([128, PB], FP32)
                    for q in range(PB // TOK):
                        off = s * S + half * PB + q * TOK
                        nc.tensor.matmul(
                            out=ps[:, q * TOK : (q + 1) * TOK],
                            lhsT=w_sb[:, (c0 + s) * 128 : (c0 + s + 1) * 128],
                            rhs=x_t[:, off : off + TOK],
                            start=True,
                            stop=True,
                        )
                    dst = o_t[:, pb * PB : (pb + 1) * PB]
                    if ncopy % 2 == 0:
                        nc.vector.tensor_copy(out=dst, in_=ps[:])
                    else:
                        nc.scalar.copy(out=dst, in_=ps[:])
                    ncopy += 1
                if ci >= len(CHUNKS) - 4:
                    # Drain: store each 512KB half as soon as its copies
                    # land, on alternating rings (loads are done; Sync
                    # ring is idle).
                    h = cw // 2
                    nc.sync.dma_start(
                        out=yp[:, c0 * S : c0 * S + h], in_=o_t[:, :h]
                    )
                    nc.scalar.dma_start(
                        out=yp[:, c0 * S + h : c0 * S + cw], in_=o_t[:, h:]
                    )
                else:
                    nc.scalar.dma_start(
                        out=yp[:, c0 * S : c0 * S + cw], in_=o_t[:]
                    )
    nc.finalize()
    return nc


def _prep_in_maps(x, W):
    # Diagonal blocks: Wdiag[g][o, i] = W[g*64+o, g*64+i]
    Wr = W.reshape(G, GS, G, GS)
    g = np.arange(G)
    WdT = Wr[g, :, g, :].transpose(0, 2, 1).astype(np.float16)    # [g, i, o]
    wb = np.zeros((128, NSTRIP, 128), dtype=np.float16)
    for c in range(NSTRIP):
        wb[0:64, c, 0:64] = WdT[2 * c]
        wb[64:128, c, 64:128] = WdT[2 * c + 1]
    wb = np.ascontiguousarray(wb.reshape(128, NSTRIP * 128))
    maps = []
    for b in range(B):
        # xp[p, c*S + t] = x[b, t, c*128 + p]
        xp = np.ascontiguousarray(
            x[b].T.reshape(NSTRIP, 128, S).transpose(1, 0, 2).reshape(128, NSTRIP * S),
            dtype=np.float16,
        )
        maps.append({"xp": xp, "wb": wb})
    return maps


def run(x, W, trace=False, **kw):
    x = np.asarray(x, dtype=np.float32)
    W = np.asarray(W, dtype=np.float32)
    nc = _build_program()
    in_maps = _prep_in_maps(x, W)
    res = run_bass_kernel_spmd(nc, in_maps, list(range(B)), trace=trace, **kw)
    y = np.empty((B, S, C), dtype=np.float32)
    for b in range(B):
        yp = res.results[b]["yp"]
        # y[b, t, c*128 + p] = yp[p, c*S + t]
        y[b] = (
            yp.reshape(128, NSTRIP, S)
            .transpose(1, 0, 2)
            .reshape(C, S)
            .T.astype(np.float32)
        )
    return y, res


def kernel(x, W):
    y, _ = run(x, W, trace=False)
    return y



# revision 4
# speedup vs baseline: 1.2505x; 1.2505x over previous
"""Block-diagonal grouped GEMM (GroupLinear) on 8 TRN2 NeuronCores, int8 I/O.

Problem: x [8, 2048, 4096] f32, W [4096, 4096] f32 where only the 64
diagonal 64x64 blocks of W are used:
    y[b,s, g*64+o] = sum_i x[b,s, g*64+i] * W[g*64+o, g*64+i]

HBM-bandwidth bound. The rel-err budget (2e-2) admits int8 transport:
  - Host quantizes x per (token, group): sx = max|x_group|/127,
    xq = round(x/sx) int8  (~0.6% rel err).
  - Weight rows are prescaled by c[g,o] = 127/(R*||Wrow||*rms(xq)) so the
    f32 PSUM accumulator lands in int8 range (R=4.5 sigma; the convert
    rounds-to-nearest and saturates -- measured on HW -- so rare clips
    are graceful). W' stays f16 (1MB).
  - Device: load xq int8 (8MB/core), DVE int8->f16 (2x dual-port mode),
    f16 matmul -> PSUM f32, convert PSUM f32 -> int8 directly, store yq
    int8 (8MB).  Host dequant: y = yq * sx / c.  ~1.2% rel err total;
    traffic 16.5MB/core vs 33MB for the f16 kernel.

Engine layout (per ~3.1us double-strip chunk in steady state):
  - Sync HWDGE ring: all x loads (emitted first; FIFO per engine).
  - GPSIMD SWDGE ring: all y stores (Scalar pays no DMA-issue time).
  - Vector: in-convert (2x) + the first OUT_V cols of each chunk's PSUM.
  - Scalar: the remaining PSUM cols (two ACTIVATE ops per chunk).
  - PSUM is one 8-bank tile; strip k uses half k%2, so out-converts of
    strip k-1 overlap matmuls of strip k on disjoint banks.
  - Emission is software-pipelined: chunk k+1's in-convert is emitted
    before chunk k's out-converts so Vector never head-of-line blocks.
  - Dep-free LDWEIGHTS fillers keep the PE activity monitor at full
    clock (idle PE throttles to 1.2GHz and doubles matmul spacing).
"""

import numpy as np

import concourse.bacc as bacc
import concourse.mybir as mybir
from concourse.tile import TileContext
from concourse.bass_utils import run_bass_kernel_spmd

B, S, C = 8, 2048, 4096
G, GS = 64, 64            # groups, group size (=in_scale=out_scale)
NSTRIP = C // 128         # 32 strips of 128 channels (2 groups each)
TOK = 512                 # matmul moving free dim (PSUM bank = 512 f32)
F16 = mybir.dt.float16
FP32 = mybir.dt.float32
I8 = mybir.dt.int8

R_SIGMA = 4.5             # accumulator headroom in sigmas before int8 clip
OUT_V = 768               # per chunk: Vector converts first OUT_V PSUM cols
NFILL = 3                 # dep-free ldweights fillers per strip (PE warmth)

# (start_strip, n_strips) DMA chunks: two single-strip leaders for a
# short pipeline fill, then double-strip chunks.
CHUNKS = [(0, 1), (1, 1)] + [(c, 2) for c in range(2, NSTRIP, 2)]


def _build_program():
    nc = bacc.Bacc()
    xp = nc.declare_dram_parameter("xp", [128, NSTRIP * S], I8, isOutput=False)
    wb = nc.declare_dram_parameter("wb", [128, NSTRIP * 128], F16, isOutput=False)
    yp = nc.declare_dram_parameter("yp", [128, NSTRIP * S], I8, isOutput=True)

    with TileContext(nc) as tc:
        with (
            tc.tile_pool(name="wpool", bufs=1) as wpool,
            tc.tile_pool(name="xlead", bufs=2) as xlead,
            tc.tile_pool(name="flead", bufs=2) as flead,
            tc.tile_pool(name="olead", bufs=2) as olead,
            tc.tile_pool(name="xpool", bufs=3) as xpool,
            tc.tile_pool(name="fpool", bufs=2) as fpool,
            tc.tile_pool(name="opool", bufs=3) as opool,
            tc.tile_pool(name="ppool", bufs=1, space="PSUM") as ppool,
        ):
            w_sb = wpool.tile([128, NSTRIP * 128], F16)
            nc.scalar.dma_start(out=w_sb[:, :256], in_=wb[:, :256])
            nc.scalar.dma_start(out=w_sb[:, 256:], in_=wb[:, 256:])

            # All loads first on the Sync ring (FIFO per engine: nothing
            # that waits on compute may sit ahead of a load).
            x_tiles = []
            for ci, (c0, ns) in enumerate(CHUNKS):
                x_t = (xlead if ci < 2 else xpool).tile([128, ns * S], I8)
                nc.sync.dma_start(
                    out=x_t[:], in_=xp[:, c0 * S : c0 * S + ns * S]
                )
                x_tiles.append(x_t)

            # One 8-bank PSUM tile; strip s uses half s%2 (banks 0-3/4-7).
            P = ppool.tile([128, 2 * S], FP32)

            def conv(ci):
                c0, ns = CHUNKS[ci]
                f_t = (flead if ci < 2 else fpool).tile([128, ns * S], F16)
                nc.vector.tensor_copy(out=f_t[:], in_=x_tiles[ci][:])
                return f_t

            f_ts = [conv(0)]

            for ci, (c0, ns) in enumerate(CHUNKS):
                cw = ns * S
                f_t = f_ts[ci]
                # Software pipeline: next chunk's in-convert is emitted
                # before this chunk's out-converts in Vector program order.
                if ci + 1 < len(CHUNKS):
                    f_ts.append(conv(ci + 1))
                o_t = (olead if ci < 2 else opool).tile([128, cw], I8)
                for j in range(ns):
                    s = c0 + j
                    H = (s % 2) * S
                    for q in range(S // TOK):
                        nc.tensor.matmul(
                            out=P[:, H + q * TOK : H + (q + 1) * TOK],
                            lhsT=w_sb[:, s * 128 : (s + 1) * 128],
                            rhs=f_t[:, j * S + q * TOK : j * S + (q + 1) * TOK],
                            start=True,
                            stop=True,
                        )
                    # Dep-free PE warmers: read-only on w_sb, no PSUM.
                    for _ in range(NFILL):
                        nc.tensor.ldweights(weights=w_sb[:, :128])
                    if j == 0 and ns == 2:
                        # Vector: first OUT_V cols of strip j=0's half.
                        nc.vector.tensor_copy(
                            out=o_t[:, :OUT_V], in_=P[:, H : H + OUT_V]
                        )
                        nc.scalar.copy(
                            out=o_t[:, OUT_V:S], in_=P[:, H + OUT_V : H + S]
                        )
                    else:
                        base = j * S
                        if ns == 1:
                            nc.vector.tensor_copy(
                                out=o_t[:, :OUT_V], in_=P[:, H : H + OUT_V]
                            )
                            nc.scalar.copy(
                                out=o_t[:, OUT_V:], in_=P[:, H + OUT_V : H + S]
                            )
                        else:
                            nc.scalar.copy(
                                out=o_t[:, base : base + S],
                                in_=P[:, H : H + S],
                            )
                if ci >= len(CHUNKS) - 2:
                    # Drain tail: split the last stores across rings.
                    h = cw // 2
                    nc.gpsimd.dma_start(
                        out=yp[:, c0 * S : c0 * S + h], in_=o_t[:, :h]
                    )
                    nc.scalar.dma_start(
                        out=yp[:, c0 * S + h : c0 * S + cw], in_=o_t[:, h:]
                    )
                else:
                    nc.gpsimd.dma_start(
                        out=yp[:, c0 * S : c0 * S + cw], in_=o_t[:]
                    )
    nc.finalize()
    return nc


def _prep_in_maps(x, W):
    # Diagonal blocks: Wdiag[g][o, i] = W[g*64+o, g*64+i]
    Wr = W.reshape(G, GS, G, GS)
    g = np.arange(G)
    Wd = Wr[g, :, g, :]                                   # [g, o, i]
    rownorm = np.linalg.norm(Wd, axis=2)                  # [g, o]
    rownorm = np.maximum(rownorm, 1e-12)

    maps = []
    dequants = []
    for b in range(B):
        xg = x[b].reshape(S, G, GS)
        sx = np.abs(xg).max(axis=2) / 127.0               # [S, G]
        sx = np.maximum(sx, 1e-30)
        xq = np.rint(xg / sx[:, :, None]).astype(np.int8)  # [S, G, GS]
        rms = float(np.sqrt(np.mean(np.square(xq.astype(np.float32)))))
        c = 127.0 / (R_SIGMA * rownorm * rms)             # [g, o]
        WdT = (Wd * c[:, :, None]).transpose(0, 2, 1).astype(np.float16)  # [g,i,o]
        wb = np.zeros((128, NSTRIP, 128), dtype=np.float16)
        for cs in range(NSTRIP):
            wb[0:64, cs, 0:64] = WdT[2 * cs]
            wb[64:128, cs, 64:128] = WdT[2 * cs + 1]
        wb = np.ascontiguousarray(wb.reshape(128, NSTRIP * 128))
        # xp[p, cs*S + t] = xq[t, channel cs*128+p] ; channel c = g*64+i
        xq_flat = xq.reshape(S, C)                        # [t, ch]
        xp = np.ascontiguousarray(
            xq_flat.T.reshape(NSTRIP, 128, S).transpose(1, 0, 2).reshape(
                128, NSTRIP * S
            )
        )
        maps.append({"xp": xp, "wb": wb})
        dequants.append((sx, 1.0 / c))
    return maps, dequants


def run(x, W, trace=False, **kw):
    x = np.asarray(x, dtype=np.float32)
    W = np.asarray(W, dtype=np.float32)
    nc = _build_program()
    in_maps, dequants = _prep_in_maps(x, W)
    res = run_bass_kernel_spmd(nc, in_maps, list(range(B)), trace=trace, **kw)
    y = np.empty((B, S, C), dtype=np.float32)
    for b in range(B):
        yp = res.results[b]["yp"]
        sx, inv_c = dequants[b]
        # y[t, cs*128 + p] = yp[p, cs*S + t] * sx[t, g] * inv_c[g, o]
        # (sx = max/127 already folds the 127)
        z = (
            yp.reshape(128, NSTRIP, S)
            .transpose(2, 1, 0)                           # [t, cs, p]
            .reshape(S, G, GS)
            .astype(np.float32)
        )
        yb = z * inv_c[None, :, :] * sx[:, :, None]
        y[b] = yb.reshape(S, C)
    return y, res


def kernel(x, W):
    y, _ = run(x, W, trace=False)
    return y


# revision 9
# speedup vs baseline: 1.3449x; 1.0755x over previous
"""Block-diagonal grouped GEMM (GroupLinear) on 8 TRN2 NeuronCores, int8 I/O.

Problem: x [8, 2048, 4096] f32, W [4096, 4096] f32 where only the 64
diagonal 64x64 blocks of W are used:
    y[b,s, g*64+o] = sum_i x[b,s, g*64+i] * W[g*64+o, g*64+i]

HBM-bandwidth bound. The rel-err budget (2e-2) admits int8 transport:
  - Host quantizes x per (token, group): sx = max|x_group|/127,
    xq = round(x/sx) int8  (~0.6% rel err).
  - Weight rows are prescaled by c[g,o] = 127/(R*||Wrow||*rms(xq)) so the
    f32 PSUM accumulator lands in int8 range (R=4.5 sigma; the convert
    rounds-to-nearest and saturates -- measured on HW -- so rare clips
    are graceful). W' stays f16 (1MB).
  - Device: load xq int8 (8MB/core), DVE int8->f16 (2x dual-port mode),
    f16 matmul -> PSUM f32, convert PSUM f32 -> int8 directly, store yq
    int8 (8MB).  Host dequant: y = yq * sx / c.  ~1.2% rel err total;
    traffic 16.5MB/core vs 33MB for the f16 kernel.

Pipeline (strip = 128 channels x 2048 tokens; 4 matmuls of N=512):
  - Sync HWDGE ring: all x loads (emitted first; FIFO per engine).
    GPSIMD SWDGE ring: all y stores (Scalar pays no DMA-issue time).
    Scalar HWDGE ring: weights only (64KB leader, then the rest).
  - PSUM is one 8-bank tile; strip s uses half s%2.  Out-converts are
    three slices -- Vector [0:OUT_V], Scalar [OUT_V:1024], [1024:2048] --
    so banks free as soon as their matmuls land and the strip s+2
    matmuls never wait on a full-strip drain.
  - Vector runs the int8->f16 in-converts strip-granular with 2-strip
    lookahead (emitted ahead of its out-slices: no head-of-line block).
  - A couple of dep-free LDWEIGHTS per strip keep the PE activity
    monitor at full clock (idle PE throttles 2.4 -> 1.2 GHz).
"""

import numpy as np

import concourse.bacc as bacc
import concourse.mybir as mybir
from concourse.tile import TileContext
from concourse.bass_utils import run_bass_kernel_spmd

B, S, C = 8, 2048, 4096
G, GS = 64, 64            # groups, group size (=in_scale=out_scale)
NSTRIP = C // 128         # 32 strips of 128 channels (2 groups each)
TOK = 512                 # matmul moving free dim (PSUM bank = 512 f32)
F16 = mybir.dt.float16
FP32 = mybir.dt.float32
I8 = mybir.dt.int8

R_SIGMA = 4.5             # accumulator headroom in sigmas before int8 clip
OUT_V = 384               # per strip: Vector converts PSUM cols [0:OUT_V]
NFILL = 2                 # dep-free ldweights fillers per strip (PE warmth)

# (start_strip, n_strips) DMA chunks: two single-strip leaders for a
# short pipeline fill, then double-strip chunks.
CHUNKS = [(0, 1), (1, 1)] + [(c, 2) for c in range(2, NSTRIP, 2)]


def _build_program():
    nc = bacc.Bacc()
    xp = nc.declare_dram_parameter("xp", [128, NSTRIP * S], I8, isOutput=False)
    wb = nc.declare_dram_parameter("wb", [128, NSTRIP * 128], F16, isOutput=False)
    yp = nc.declare_dram_parameter("yp", [128, NSTRIP * S], I8, isOutput=True)

    # strip index -> (chunk index, offset-within-chunk)
    strip_loc = {}
    for ci, (c0, ns) in enumerate(CHUNKS):
        for j in range(ns):
            strip_loc[c0 + j] = (ci, j)

    with TileContext(nc) as tc:
        with (
            tc.tile_pool(name="wpool", bufs=1) as wpool,
            tc.tile_pool(name="xlead", bufs=2) as xlead,
            tc.tile_pool(name="olead", bufs=2) as olead,
            tc.tile_pool(name="xpool", bufs=3) as xpool,
            tc.tile_pool(name="fpool", bufs=4) as fpool,
            tc.tile_pool(name="opool", bufs=3) as opool,
            tc.tile_pool(name="ppool", bufs=1, space="PSUM") as ppool,
        ):
            w_sb = wpool.tile([128, NSTRIP * 128], F16)
            nc.scalar.dma_start(out=w_sb[:, :512], in_=wb[:, :512])
            nc.scalar.dma_start(out=w_sb[:, 512:], in_=wb[:, 512:])

            # All loads first on the Sync ring (FIFO per engine: nothing
            # that waits on compute may sit ahead of a load).
            x_tiles = []
            for ci, (c0, ns) in enumerate(CHUNKS):
                x_t = (xlead if ci < 2 else xpool).tile([128, ns * S], I8)
                nc.sync.dma_start(
                    out=x_t[:], in_=xp[:, c0 * S : c0 * S + ns * S]
                )
                x_tiles.append(x_t)

            # One 8-bank PSUM tile; strip s uses half s%2 (banks 0-3/4-7).
            P = ppool.tile([128, 2 * S], FP32)

            def conv(s):
                ci, j = strip_loc[s]
                f_t = fpool.tile([128, S], F16)
                nc.vector.tensor_copy(
                    out=f_t[:], in_=x_tiles[ci][:, j * S : (j + 1) * S]
                )
                return f_t

            f_ts = [conv(0), conv(1)]

            o_tiles = {}

            for s in range(NSTRIP):
                ci, j = strip_loc[s]
                c0, ns = CHUNKS[ci]
                if j == 0:
                    o_tiles[ci] = (olead if ci < 2 else opool).tile(
                        [128, ns * S], I8, name="o_t"
                    )
                f_t = f_ts[s]
                # Vector lookahead: in-convert strip s+2 ahead of this
                # strip's out-slice in Vector program order.
                if s + 2 < NSTRIP:
                    f_ts.append(conv(s + 2))
                H = (s % 2) * S
                for q in range(S // TOK):
                    nc.tensor.matmul(
                        out=P[:, H + q * TOK : H + (q + 1) * TOK],
                        lhsT=w_sb[:, s * 128 : (s + 1) * 128],
                        rhs=f_t[:, q * TOK : (q + 1) * TOK],
                        start=True,
                        stop=True,
                    )
                for _ in range(NFILL):
                    nc.tensor.ldweights(weights=w_sb[:, :128])
                o_t = o_tiles[ci]
                base = j * S
                nc.vector.tensor_copy(
                    out=o_t[:, base : base + OUT_V], in_=P[:, H : H + OUT_V]
                )
                nc.scalar.copy(
                    out=o_t[:, base + OUT_V : base + 1024],
                    in_=P[:, H + OUT_V : H + 1024],
                )
                nc.scalar.copy(
                    out=o_t[:, base + 1024 : base + S],
                    in_=P[:, H + 1024 : H + S],
                )
                if j == ns - 1:
                    cw = ns * S
                    if ci >= len(CHUNKS) - 2:
                        h = cw // 2
                        nc.gpsimd.dma_start(
                            out=yp[:, c0 * S : c0 * S + h], in_=o_t[:, :h]
                        )
                        nc.scalar.dma_start(
                            out=yp[:, c0 * S + h : c0 * S + cw], in_=o_t[:, h:]
                        )
                    else:
                        nc.gpsimd.dma_start(
                            out=yp[:, c0 * S : c0 * S + cw], in_=o_t[:]
                        )
    nc.finalize()
    return nc


def _prep_in_maps(x, W):
    # Diagonal blocks: Wdiag[g][o, i] = W[g*64+o, g*64+i]
    Wr = W.reshape(G, GS, G, GS)
    g = np.arange(G)
    Wd = Wr[g, :, g, :]                                   # [g, o, i]
    rownorm = np.linalg.norm(Wd, axis=2)                  # [g, o]
    rownorm = np.maximum(rownorm, 1e-12)

    maps = []
    dequants = []
    for b in range(B):
        xg = x[b].reshape(S, G, GS)
        sx = np.abs(xg).max(axis=2) / 127.0               # [S, G]
        sx = np.maximum(sx, 1e-30)
        xq = np.rint(xg / sx[:, :, None]).astype(np.int8)  # [S, G, GS]
        rms = float(np.sqrt(np.mean(np.square(xq.astype(np.float32)))))
        c = 127.0 / (R_SIGMA * rownorm * rms)             # [g, o]
        WdT = (Wd * c[:, :, None]).transpose(0, 2, 1).astype(np.float16)  # [g,i,o]
        wb = np.zeros((128, NSTRIP, 128), dtype=np.float16)
        for cs in range(NSTRIP):
            wb[0:64, cs, 0:64] = WdT[2 * cs]
            wb[64:128, cs, 64:128] = WdT[2 * cs + 1]
        wb = np.ascontiguousarray(wb.reshape(128, NSTRIP * 128))
        # xp[p, cs*S + t] = xq[t, channel cs*128+p] ; channel c = g*64+i
        xq_flat = xq.reshape(S, C)                        # [t, ch]
        xp = np.ascontiguousarray(
            xq_flat.T.reshape(NSTRIP, 128, S).transpose(1, 0, 2).reshape(
                128, NSTRIP * S
            )
        )
        maps.append({"xp": xp, "wb": wb})
        dequants.append((sx, 1.0 / c))
    return maps, dequants


def run(x, W, trace=False, **kw):
    x = np.asarray(x, dtype=np.float32)
    W = np.asarray(W, dtype=np.float32)
    nc = _build_program()
    in_maps, dequants = _prep_in_maps(x, W)
    res = run_bass_kernel_spmd(nc, in_maps, list(range(B)), trace=trace, **kw)
    y = np.empty((B, S, C), dtype=np.float32)
    for b in range(B):
        yp = res.results[b]["yp"]
        sx, inv_c = dequants[b]
        # y[t, cs*128 + p] = yp[p, cs*S + t] * sx[t, g] * inv_c[g, o]
        # (sx = max/127 already folds the 127)
        z = (
            yp.reshape(128, NSTRIP, S)
            .transpose(2, 1, 0)                           # [t, cs, p]
            .reshape(S, G, GS)
            .astype(np.float32)
        )
        yb = z * inv_c[None, :, :] * sx[:, :, None]
        y[b] = yb.reshape(S, C)
    return y, res


def kernel(x, W):
    y, _ = run(x, W, trace=False)
    return y
